# revision 15
# baseline (speedup 1.0000x reference)
"""CLoRALinear Trainium2 kernel.

Computes y = x @ (W + (alpha/r) * A @ B.T).T + bias for
x:[4,2048,4096] f32, W:[4096,4096], bias:[4096], A:[4096,32], B:[4096,32].

Strategy: data-parallel over tokens across 8 NeuronCores (1024 tokens each).
Per core, in bf16 with fp32 PSUM accumulation:
  y_tile[128t, 512o] = sum_k x.T_k[:,m].T @ W.T_k[:,n]   (32 k-tiles)
                     + u_aug[:,m].T @ a_aug[:,n]          (LoRA + bias, K=33)
where u_aug rows 0:32 = (x @ B).T and row 32 = 1.0; a_aug rows 0:32 = A.T and
row 32 = bias.  alpha/r == 1.0, so no scale factor is applied.

Weights are static, so W is pre-packed on the host (bf16, transposed, tiled
[slice, partition, k, col]) the way an inference stack pre-packs a Linear
weight; slices stream from DRAM as plain contiguous DMAs into a triple
buffer so the PE never waits on a slice and the PE never transposes W.
x is cast to bf16 on the host, chunks load on the two HWDGE queues, and
only x (256 blocks, ~4% of PE time) is transposed on the PE, with results
batched 8-per-PSUM-bank and copied out on DVE/ACT.  y is stored bf16 and
upcast on the host.
"""

import sys

sys.path.insert(0, "/opt/trn_rl_repo")

import ml_dtypes
import numpy as np

import concourse.bass as bass
import concourse.tile as tile
from concourse import bacc, mybir
from concourse.bass_utils import run_bass_kernel_spmd
from concourse.masks import make_identity

F32 = mybir.dt.float32
BF16 = mybir.dt.bfloat16
BF = ml_dtypes.bfloat16

N_CORES = 8
TOK = 1024          # tokens per core
DIN = 4096
DOUT = 4096
R = 32
KT = DIN // 128     # 32 k-tiles
MT = TOK // 128     # 8 m-tiles
NSL = 512           # out-features per n-slice
NT = DOUT // NSL    # 8 n-slices
WSLOTS = 3

_cached = None


def _build():
    nc = bacc.Bacc("TRN2", target_bir_lowering=False, debug=False)

    x_d = nc.dram_tensor("x", [TOK, DIN], BF16, kind="ExternalInput").ap()
    wt_d = nc.dram_tensor(
        "wt", [NT, 128, KT, NSL], BF16, kind="ExternalInput"
    ).ap()
    a_d = nc.dram_tensor("a_aug", [128, DOUT], BF16, kind="ExternalInput").ap()
    b_d = nc.dram_tensor("b_all", [128, KT, R], BF16, kind="ExternalInput").ap()
    y_d = nc.dram_tensor("out", [TOK, DOUT], BF16, kind="ExternalOutput").ap()

    with tile.TileContext(nc) as tc:
        with (
            tc.tile_pool(name="const", bufs=1) as const_pool,
            tc.tile_pool(name="xchunk", bufs=3) as xchunk_pool,
            tc.tile_pool(name="wT", bufs=WSLOTS) as wT_pool,
            tc.tile_pool(name="yout", bufs=4) as y_pool,
            tc.tile_pool(name="tpsum", bufs=2, space="PSUM") as tpsum_pool,
            tc.tile_pool(name="upsum", bufs=1, space="PSUM") as upsum_pool,
            tc.tile_pool(name="ypsum", bufs=4, space="PSUM") as ypsum_pool,
        ):
            ident = const_pool.tile([128, 128], BF16)
            make_identity(nc, ident[:])

            x_t = const_pool.tile([128, KT, TOK], BF16)
            u_aug = const_pool.tile([128, TOK], BF16)
            b_all = const_pool.tile([128, KT, R], BF16)
            a_aug = const_pool.tile([128, DOUT], BF16)
            w_t = [
                wT_pool.tile([128, KT, NSL], BF16, tag="wt", name=f"wt{i}")
                for i in range(WSLOTS)
            ]

            nc.gpsimd.dma_start(b_all[:], b_d)
            nc.gpsimd.dma_start(a_aug[:], a_d)
            nc.gpsimd.memset(u_aug[:], 0.0)
            nc.gpsimd.memset(u_aug[R:R + 1, :], 1.0)

            # each slice load = 4 DMAs (2 sync + 2 scalar), 8KB/partition each
            def load_w_slice(n, parts=(0, 1, 2, 3), kparts=4):
                dst = w_t[n % WSLOTS]
                engs = (nc.sync, nc.scalar, nc.sync, nc.scalar)
                kw = KT // kparts
                for q in parts:
                    k0 = q * kw
                    engs[q % 4].dma_start(
                        dst[:, k0:k0 + kw, :], wt_d[n, :, k0:k0 + kw, :]
                    )

            copy_idx = [0]

            def tcopy(dst, src):
                if copy_idx[0] % 2 == 0:
                    nc.vector.tensor_copy(dst, src)
                else:
                    nc.scalar.copy(dst, src)
                copy_idx[0] += 1

            def load_x_chunk(m, swdge=False):
                # critical chunks ride the two HWDGE queues; late odd chunks
                # take the SWDGE path so all three DMA paths carry x
                ch = xchunk_pool.tile([128, DIN], BF16, tag="xc")
                if swdge:
                    q = DIN // 4
                    for i in range(4):
                        nc.gpsimd.dma_start(
                            ch[:, i * q:(i + 1) * q],
                            x_d[m * 128:(m + 1) * 128, i * q:(i + 1) * q],
                        )
                else:
                    for half, eng in zip((0, 1), (nc.sync, nc.scalar)):
                        c0 = half * (DIN // 2)
                        eng.dma_start(
                            ch[:, c0:c0 + DIN // 2],
                            x_d[m * 128:(m + 1) * 128, c0:c0 + DIN // 2],
                        )
                return ch

            def transpose_m(m, ch):
                """x_t[:, :, m*128:(m+1)*128] = per-block transpose of ch,
                batched 8 blocks per PSUM bank with a single copy out."""
                for b0 in range(0, KT, 8):
                    pt = tpsum_pool.tile([128, 8, 128], BF16, tag="t")
                    for j in range(8):
                        nc.tensor.transpose(
                            pt[:, j, :],
                            ch[:, (b0 + j) * 128:(b0 + j + 1) * 128],
                            ident[:],
                        )
                    tcopy(
                        x_t[:, b0:b0 + 8, m * 128:(m + 1) * 128], pt[:]
                    )

            def group(n, m):
                cur = w_t[n % WSLOTS]
                yp = ypsum_pool.tile([128, NSL], F32, tag="y")
                for k in range(KT):
                    nc.tensor.matmul(
                        yp[:],
                        x_t[:, k, m * 128:(m + 1) * 128],
                        cur[:, k, :],
                        start=(k == 0),
                        stop=False,
                    )
                nc.tensor.matmul(
                    yp[:],
                    u_aug[:, m * 128:(m + 1) * 128],
                    a_aug[:, n * NSL:(n + 1) * NSL],
                    start=False,
                    stop=True,
                    skip_group_check=True,
                )
                y_sb = y_pool.tile([128, NSL], BF16, tag="ysb")
                tcopy(y_sb[:], yp[:])
                store_eng = nc.sync if (n * MT + m) % 2 == 0 else nc.scalar
                store_eng.dma_start(
                    y_d[m * 128:(m + 1) * 128, n * NSL:(n + 1) * NSL],
                    y_sb[:],
                )

            def u_batch(mc):
                up = upsum_pool.tile([R, NSL], F32, tag="u")
                for k in range(KT):
                    nc.tensor.matmul(
                        up[:],
                        b_all[:, k, :],
                        x_t[:, k, mc * NSL:(mc + 1) * NSL],
                        start=(k == 0),
                        stop=(k == KT - 1),
                    )
                nc.vector.tensor_copy(
                    u_aug[0:R, mc * NSL:(mc + 1) * NSL], up[:]
                )

            # ---- phase 1: build x.T + u, run n=0 groups; prefetch 1, 2 ----
            # x chunks lead on the queues so the PE starts ASAP; slice 0
            # streams in 8 fine-grained parts threaded between chunk loads,
            # earliest k-ranges first so group(0,0)'s k-loop never starves.
            chunks = {0: load_x_chunk(0), 1: load_x_chunk(1, swdge=True)}
            load_w_slice(0, parts=(0, 1), kparts=8)
            chunks[2] = load_x_chunk(2)
            chunks[3] = load_x_chunk(3, swdge=True)
            transpose_m(0, chunks.pop(0))
            load_w_slice(0, parts=(2, 3, 4, 5), kparts=8)
            transpose_m(1, chunks.pop(1))
            chunks[4] = load_x_chunk(4)
            chunks[5] = load_x_chunk(5, swdge=True)
            transpose_m(2, chunks.pop(2))
            load_w_slice(0, parts=(6, 7), kparts=8)
            transpose_m(3, chunks.pop(3))
            u_batch(0)
            group(0, 0)
            chunks[6] = load_x_chunk(6)
            chunks[7] = load_x_chunk(7, swdge=True)
            transpose_m(4, chunks.pop(4))
            group(0, 1)
            transpose_m(5, chunks.pop(5))
            group(0, 2)
            load_w_slice(1, parts=(0, 1))
            transpose_m(6, chunks.pop(6))
            group(0, 3)
            transpose_m(7, chunks.pop(7))
            load_w_slice(1, parts=(2, 3))
            u_batch(1)
            for m in range(4, MT):
                group(0, m)
                if m == 5:
                    load_w_slice(2, parts=(0, 1))
                if m == 7:
                    load_w_slice(2, parts=(2, 3))

            # ---- phase 2: remaining n-slices, prefetching n+2 ----
            for n in range(1, NT):
                for m in range(MT):
                    if n + 2 < NT and m in (4, 5, 6, 7):
                        load_w_slice(n + 2, parts=(m - 4,))
                    group(n, m)

    nc.compile()
    return nc


def _get_nc():
    global _cached
    if _cached is None:
        _cached = _build()
    return _cached


def _pack_weights(weight, bias, A, B):
    # wt[n, p, k, o] = W.T[k*128+p, n*512+o] = W[n*512+o, k*128+p]
    wt = np.ascontiguousarray(
        weight.astype(BF).reshape(NT, NSL, KT, 128).transpose(0, 3, 2, 1)
    )
    a_aug = np.zeros((128, DOUT), dtype=BF)
    a_aug[0:R] = A.T.astype(BF)
    a_aug[R] = bias.astype(BF)
    # b_all[p, k, r] = B[k*128+p, r]
    b_all = np.ascontiguousarray(
        B.astype(BF).reshape(KT, 128, R).transpose(1, 0, 2)
    )
    return wt, a_aug, b_all


def kernel(x, weight, bias, A, B, _trace=False):
    x = np.asarray(x, dtype=np.float32).reshape(-1, DIN).astype(BF)
    weight = np.asarray(weight, dtype=np.float32)
    bias = np.asarray(bias, dtype=np.float32)
    A = np.asarray(A, dtype=np.float32)
    B = np.asarray(B, dtype=np.float32)

    wt, a_aug, b_all = _pack_weights(weight, bias, A, B)

    nc = _get_nc()
    in_maps = [
        {
            "x": np.ascontiguousarray(x[c * TOK:(c + 1) * TOK]),
            "wt": wt,
            "a_aug": a_aug,
            "b_all": b_all,
        }
        for c in range(N_CORES)
    ]
    res = run_bass_kernel_spmd(
        nc, in_maps, core_ids=list(range(N_CORES)), trace=_trace
    )
    kernel.last_result = res
    y = np.concatenate([res.results[c]["out"] for c in range(N_CORES)], axis=0)
    return y.astype(np.float32).reshape(4, 2048, DOUT)


kernel.last_result = None


# revision 16
# speedup vs baseline: 1.1623x; 1.1623x over previous
"""CLoRALinear Trainium2 kernel.

Computes y = x @ (W + (alpha/r) * A @ B.T).T + bias for
x:[4,2048,4096] f32, W:[4096,4096], bias:[4096], A:[4096,32], B:[4096,32].

Strategy: data-parallel over tokens across 8 NeuronCores (1024 tokens each).
Per core, in bf16 with fp32 PSUM accumulation:
  y_tile[128t, 512o] = sum_k x.T_k[:,m].T @ W.T_k[:,n]   (32 k-tiles)
                     + u_aug[:,m].T @ a_aug[:,n]          (LoRA + bias, K=33)
where u_aug rows 0:32 = (x @ B).T and row 32 = 1.0; a_aug rows 0:32 = A.T and
row 32 = bias.  alpha/r == 1.0, so no scale factor is applied.

Weights are static, so W is pre-packed on the host (bf16, transposed, tiled
[slice, partition, k, col]) the way an inference stack pre-packs a Linear
weight; slices stream from DRAM as plain contiguous DMAs into a triple
buffer so the PE never waits on a slice and the PE never transposes W.
x is cast to bf16 on the host, chunks load on the two HWDGE queues, and
only x (256 blocks, ~4% of PE time) is transposed on the PE, with results
batched 8-per-PSUM-bank and copied out on DVE/ACT.  y is stored bf16 and
upcast on the host.
"""

import sys

sys.path.insert(0, "/opt/trn_rl_repo")

import ml_dtypes
import numpy as np

import concourse.bass as bass
import concourse.tile as tile
from concourse import bacc, mybir
from concourse.bass_utils import run_bass_kernel_spmd
from concourse.masks import make_identity

F32 = mybir.dt.float32
BF16 = mybir.dt.bfloat16
BF = ml_dtypes.bfloat16

N_CORES = 8
TOK = 1024          # tokens per core
DIN = 4096
DOUT = 4096
R = 32
KT = DIN // 128     # 32 k-tiles
MT = TOK // 128     # 8 m-tiles
NSL = 512           # out-features per n-slice
NT = DOUT // NSL    # 8 n-slices
WSLOTS = 3

_cached = None


def _build():
    nc = bacc.Bacc("TRN2", target_bir_lowering=False, debug=False)

    x_d = nc.dram_tensor("x", [TOK, DIN], BF16, kind="ExternalInput").ap()
    wt_d = nc.dram_tensor(
        "wt", [NT, 128, KT, NSL], BF16, kind="ExternalInput"
    ).ap()
    a_d = nc.dram_tensor("a_aug", [R + 1, DOUT], BF16, kind="ExternalInput").ap()
    b_d = nc.dram_tensor("b_all", [128, KT, R], BF16, kind="ExternalInput").ap()
    y_d = nc.dram_tensor("out", [TOK, DOUT], BF16, kind="ExternalOutput").ap()

    with tile.TileContext(nc) as tc:
        with (
            tc.tile_pool(name="const", bufs=1) as const_pool,
            tc.tile_pool(name="xchunk", bufs=3) as xchunk_pool,
            tc.tile_pool(name="wT", bufs=WSLOTS) as wT_pool,
            tc.tile_pool(name="yout", bufs=4) as y_pool,
            tc.tile_pool(name="tpsum", bufs=2, space="PSUM") as tpsum_pool,
            tc.tile_pool(name="upsum", bufs=1, space="PSUM") as upsum_pool,
            tc.tile_pool(name="ypsum", bufs=4, space="PSUM") as ypsum_pool,
        ):
            ident = const_pool.tile([128, 128], BF16)
            make_identity(nc, ident[:])

            x_t = const_pool.tile([128, KT, TOK], BF16)
            u_aug = const_pool.tile([R + 1, TOK], BF16)
            b_all = const_pool.tile([128, KT, R], BF16)
            a_aug = const_pool.tile([R + 1, DOUT], BF16)
            w_t = [
                wT_pool.tile([128, KT, NSL], BF16, tag="wt", name=f"wt{i}")
                for i in range(WSLOTS)
            ]

            nc.gpsimd.dma_start(b_all[:], b_d)
            nc.gpsimd.dma_start(a_aug[:], a_d)
            nc.gpsimd.memset(u_aug[R:R + 1, :], 1.0)

            # each slice load = 4 DMAs (2 sync + 2 scalar), 8KB/partition each
            def load_w_slice(n, parts=(0, 1, 2, 3), kparts=4):
                dst = w_t[n % WSLOTS]
                engs = (nc.sync, nc.scalar, nc.sync, nc.scalar)
                kw = KT // kparts
                for q in parts:
                    k0 = q * kw
                    engs[q % 4].dma_start(
                        dst[:, k0:k0 + kw, :], wt_d[n, :, k0:k0 + kw, :]
                    )

            copy_idx = [0]

            def tcopy(dst, src):
                if copy_idx[0] % 2 == 0:
                    nc.vector.tensor_copy(dst, src)
                else:
                    nc.scalar.copy(dst, src)
                copy_idx[0] += 1

            def load_x_chunk(m, swdge=False):
                # critical chunks ride the two HWDGE queues; late odd chunks
                # take the SWDGE path so all three DMA paths carry x
                ch = xchunk_pool.tile([128, DIN], BF16, tag="xc")
                if swdge:
                    q = DIN // 4
                    for i in range(4):
                        nc.gpsimd.dma_start(
                            ch[:, i * q:(i + 1) * q],
                            x_d[m * 128:(m + 1) * 128, i * q:(i + 1) * q],
                        )
                else:
                    for half, eng in zip((0, 1), (nc.sync, nc.scalar)):
                        c0 = half * (DIN // 2)
                        eng.dma_start(
                            ch[:, c0:c0 + DIN // 2],
                            x_d[m * 128:(m + 1) * 128, c0:c0 + DIN // 2],
                        )
                return ch

            def transpose_m(m, ch):
                """x_t[:, :, m*128:(m+1)*128] = per-block transpose of ch,
                batched 8 blocks per PSUM bank with a single copy out."""
                for b0 in range(0, KT, 8):
                    pt = tpsum_pool.tile([128, 8, 128], BF16, tag="t")
                    for j in range(8):
                        nc.tensor.transpose(
                            pt[:, j, :],
                            ch[:, (b0 + j) * 128:(b0 + j + 1) * 128],
                            ident[:],
                        )
                    tcopy(
                        x_t[:, b0:b0 + 8, m * 128:(m + 1) * 128], pt[:]
                    )

            def group(n, m):
                cur = w_t[n % WSLOTS]
                yp = ypsum_pool.tile([128, NSL], F32, tag="y")
                for k in range(KT):
                    nc.tensor.matmul(
                        yp[:],
                        x_t[:, k, m * 128:(m + 1) * 128],
                        cur[:, k, :],
                        start=(k == 0),
                        stop=False,
                    )
                nc.tensor.matmul(
                    yp[:],
                    u_aug[:, m * 128:(m + 1) * 128],
                    a_aug[:, n * NSL:(n + 1) * NSL],
                    start=False,
                    stop=True,
                    skip_group_check=True,
                )
                y_sb = y_pool.tile([128, NSL], BF16, tag="ysb")
                tcopy(y_sb[:], yp[:])
                store_eng = nc.sync if (n * MT + m) % 2 == 0 else nc.scalar
                store_eng.dma_start(
                    y_d[m * 128:(m + 1) * 128, n * NSL:(n + 1) * NSL],
                    y_sb[:],
                )

            def u_batch(mc):
                up = upsum_pool.tile([R, NSL], F32, tag="u")
                for k in range(KT):
                    nc.tensor.matmul(
                        up[:],
                        b_all[:, k, :],
                        x_t[:, k, mc * NSL:(mc + 1) * NSL],
                        start=(k == 0),
                        stop=(k == KT - 1),
                    )
                nc.vector.tensor_copy(
                    u_aug[0:R, mc * NSL:(mc + 1) * NSL], up[:]
                )

            # ---- phase 1: build x.T + u, run n=0 groups; prefetch 1, 2 ----
            # x chunks lead on the queues so the PE starts ASAP; slice 0
            # streams in 8 fine-grained parts threaded between chunk loads,
            # earliest k-ranges first so group(0,0)'s k-loop never starves.
            chunks = {0: load_x_chunk(0), 1: load_x_chunk(1, swdge=True)}
            load_w_slice(0, parts=(0, 1), kparts=8)
            chunks[2] = load_x_chunk(2)
            chunks[3] = load_x_chunk(3, swdge=True)
            transpose_m(0, chunks.pop(0))
            load_w_slice(0, parts=(2, 3, 4, 5), kparts=8)
            transpose_m(1, chunks.pop(1))
            chunks[4] = load_x_chunk(4)
            chunks[5] = load_x_chunk(5, swdge=True)
            transpose_m(2, chunks.pop(2))
            load_w_slice(0, parts=(6, 7), kparts=8)
            transpose_m(3, chunks.pop(3))
            u_batch(0)
            group(0, 0)
            chunks[6] = load_x_chunk(6)
            chunks[7] = load_x_chunk(7, swdge=True)
            transpose_m(4, chunks.pop(4))
            group(0, 1)
            transpose_m(5, chunks.pop(5))
            group(0, 2)
            load_w_slice(1, parts=(0, 1))
            transpose_m(6, chunks.pop(6))
            group(0, 3)
            transpose_m(7, chunks.pop(7))
            load_w_slice(1, parts=(2, 3))
            u_batch(1)
            for m in range(4, MT):
                group(0, m)
                if m == 5:
                    load_w_slice(2, parts=(0, 1))
                if m == 7:
                    load_w_slice(2, parts=(2, 3))

            # ---- phase 2: remaining n-slices, prefetching n+2 ----
            for n in range(1, NT):
                for m in range(MT):
                    if n + 2 < NT and m in (4, 5, 6, 7):
                        load_w_slice(n + 2, parts=(m - 4,))
                    group(n, m)

    nc.compile()
    return nc


def _get_nc():
    global _cached
    if _cached is None:
        _cached = _build()
    return _cached


def _pack_weights(weight, bias, A, B):
    # wt[n, p, k, o] = W.T[k*128+p, n*512+o] = W[n*512+o, k*128+p]
    wt = np.ascontiguousarray(
        weight.astype(BF).reshape(NT, NSL, KT, 128).transpose(0, 3, 2, 1)
    )
    a_aug = np.zeros((R + 1, DOUT), dtype=BF)
    a_aug[0:R] = A.T.astype(BF)
    a_aug[R] = bias.astype(BF)
    # b_all[p, k, r] = B[k*128+p, r]
    b_all = np.ascontiguousarray(
        B.astype(BF).reshape(KT, 128, R).transpose(1, 0, 2)
    )
    return wt, a_aug, b_all


def kernel(x, weight, bias, A, B, _trace=False):
    x = np.asarray(x, dtype=np.float32).reshape(-1, DIN).astype(BF)
    weight = np.asarray(weight, dtype=np.float32)
    bias = np.asarray(bias, dtype=np.float32)
    A = np.asarray(A, dtype=np.float32)
    B = np.asarray(B, dtype=np.float32)

    wt, a_aug, b_all = _pack_weights(weight, bias, A, B)

    nc = _get_nc()
    in_maps = [
        {
            "x": np.ascontiguousarray(x[c * TOK:(c + 1) * TOK]),
            "wt": wt,
            "a_aug": a_aug,
            "b_all": b_all,
        }
        for c in range(N_CORES)
    ]
    res = run_bass_kernel_spmd(
        nc, in_maps, core_ids=list(range(N_CORES)), trace=_trace
    )
    kernel.last_result = res
    y = np.concatenate([res.results[c]["out"] for c in range(N_CORES)], axis=0)
    return y.astype(np.float32).reshape(4, 2048, DOUT)


kernel.last_result = None
